# revision 66
# baseline (speedup 1.0000x reference)
"""Distributed Bass kernel for nn_AttentionCircuit (B=2,S=2048,D=2048,RANK=512,H=16).

Sharding: 8 cores = 2 batches x 4 head-groups (4 heads / 512 D-cols each).
All matmuls in bf16 (1 cycle/row on PE, half the DMA/SBUF of fp32).

Per-core dataflow (contraction always on the partition axis, no on-device
transposes; host pre-transposes x / gates):
  AB (fused, one pass over x^T, s-chunks of 512):
     t_qk^T = qk_read @ x^T ; gate -> Qg^T, Kg^T   (bf16)
     t_v^T  = v_read  @ x^T ; gate -> Vg^T
     Q^T/K^T = qk_write_hg.T @ {Q,K}g^T   (transposed [d',s], bf16 SBUF)
     V       = Vg^T.T @ v_write_hg        (natural [s,d'], bf16 SBUF)
  C: per 512-query chunk t, per head: scores^T = K^T.T Q^T -> exp (bf16, no
     max-sub; logits are tiny) -> causal mask (block-skip + diag masks) ->
     replicated rowsum via ones128-matmul -> PV matmul -> ao = pv * recip
     -> AllGather(group of 4, bf16) overlapped with next chunks
  D: out_cols = AO_full^T.T @ W_O[:,cols], chunks interleaved after AG(t)
     completes so the readback DMA spreads across phase C.
"""
import sys
import numpy as np
import ml_dtypes

sys.path.insert(0, '/opt/trn_rl_repo')

import concourse.bass as bass  # noqa: E402
from concourse import bacc  # noqa: E402
import concourse.mybir as mybir  # noqa: E402
import concourse.tile as tile  # noqa: E402
from concourse.bass_utils import run_bass_kernel_spmd  # noqa: E402

B, S, D = 2, 2048, 2048
RANK = 512
NH = 16
HG = 4              # head-groups == cores per batch
DHG = D // HG       # 512 cols per head-group (4 heads)
DH = D // NH        # 128 head dim
P = 128
DB = D // P         # 16 d-blocks
RB = RANK // P      # 4 rank-blocks
SB = S // P         # 16 s-blocks
NT = S // 512       # 4 si tiles of 512
SC = 512            # fused-A s-chunk width
NSC = S // SC       # 4

F32 = mybir.dt.float32
BF16 = mybir.dt.bfloat16
F8 = mybir.dt.float8e4
DR = mybir.MatmulPerfMode.DoubleRow
AF = mybir.ActivationFunctionType
ALU = mybir.AluOpType

QK_PRESCALE = 16.0  # host scales qk_read AND qk_write by 16 into fp8 range
EXP_SCALE = 1.0 / float(np.sqrt(DH)) / QK_PRESCALE ** 4
KEEP2_Q = 0.8125    # fp8e4m3(0.81); exact residual folded into wo host-side
KEEP2_BF = float(np.float32(ml_dtypes.bfloat16(0.81)))  # ones128 constant as hw sees it
RGROUPS = [[0, 1, 2, 3], [4, 5, 6, 7]]

_CACHE = {}


def _r(ap):
    """[ (o p), f ] DRAM tensor -> [p, o, f] partition-tiled view."""
    return ap.rearrange("(o p) f -> p o f", p=P)


def _build():
    nc = bacc.Bacc("TRN2", target_bir_lowering=False, debug=False,
                   enable_asserts=False, num_devices=8)
    xT = nc.dram_tensor("xT", [D, S], BF16, kind="ExternalInput").ap()
    xT8 = nc.dram_tensor("xT8", [D, S], F8, kind="ExternalInput").ap()
    gqT = nc.dram_tensor("gqT", [RANK, S], BF16, kind="ExternalInput").ap()
    gkT = nc.dram_tensor("gkT", [RANK, S], BF16, kind="ExternalInput").ap()
    gvT = nc.dram_tensor("gvT", [RANK, S], BF16, kind="ExternalInput").ap()
    qk_readT8 = nc.dram_tensor("qk_readT8", [D, RANK], F8, kind="ExternalInput").ap()
    v_readT = nc.dram_tensor("v_readT", [D, RANK], BF16, kind="ExternalInput").ap()
    qk_w = nc.dram_tensor("qk_write_hg", [RANK, DHG], F8, kind="ExternalInput").ap()
    v_w = nc.dram_tensor("v_write_hg", [RANK, DHG], BF16, kind="ExternalInput").ap()
    wo = nc.dram_tensor("wo_cols", [D, DHG], BF16, kind="ExternalInput").ap()
    out = nc.dram_tensor("out", [S, DHG], F32, kind="ExternalOutput").ap()

    with tile.TileContext(nc) as tc:
        _body(tc, xT, xT8, gqT, gkT, gvT, qk_readT8, v_readT, qk_w, v_w, wo,
              out)
    nc.compile()
    return nc


def _body(tc, xT, xT8, gqT, gkT, gvT, qk_readT8, v_readT, qk_w, v_w, wo, out):
    nc = tc.nc
    import contextlib
    ctx = contextlib.ExitStack()
    with ctx:
        # ---- long-lived activation tensors
        pool_qk = ctx.enter_context(tc.tile_pool(name="qk", bufs=1))
        QT_sb = pool_qk.tile([P, HG, S], BF16)          # Q^T [d', s]
        KT_sb = pool_qk.tile([P, HG, S], BF16)
        V_sb = pool_qk.tile([P, SB, DHG], BF16)         # V natural [s, d']

        # warmup AllGather at sim-time 0: pays the first-collective setup
        # cost during phase AB instead of in front of chunk 0's AG; sized
        # 128KB so the data path (not just control) is warmed
        pool_warm = ctx.enter_context(tc.tile_pool(name="warm", bufs=1))
        with tc.high_priority():
            warm_sb = pool_warm.tile([P, 512], BF16)
            nc.vector.memset(warm_sb[:], 0.0)
            warm_in = pool_warm.tile([P, 512], BF16, space="DRAM")
            warm_out = pool_warm.tile([4 * P, 512], BF16, space="DRAM")
            nc.sync.dma_start(warm_in[:], warm_sb[:])
            nc.gpsimd.collective_compute(
                "AllGather", ALU.bypass, ins=[warm_in[:].opt()],
                outs=[warm_out[:].opt()], replica_groups=RGROUPS)

        # ========== fused A+B per s-chunk: Q^T, K^T, V ==========
        with (
            tc.tile_pool(name="rd", bufs=1) as pool_rd,
            tc.tile_pool(name="wr", bufs=1) as pool_wr,
            tc.tile_pool(name="ax", bufs=2) as pool_x,
            tc.tile_pool(name="ag", bufs=2) as pool_g,
            tc.tile_pool(name="gch", bufs=2) as pool_gch,
            tc.tile_pool(name="psA", bufs=2, space="PSUM") as psA,
            tc.tile_pool(name="psB", bufs=2, space="PSUM") as psB,
        ):
            qr8_sb = pool_rd.tile([P, DB, RANK], F8)
            vr_sb = pool_rd.tile([P, DB, RANK], BF16)
            qw8_sb = pool_wr.tile([P, RB, DHG], F8)
            vw_sb = pool_wr.tile([P, RB, DHG], BF16)
            xt0 = pool_x.tile([P, DB, SC], BF16, tag="xt")
            x80 = pool_x.tile([P, DB, SC], F8, tag="x8")
            gq0 = pool_g.tile([P, RB, SC], BF16, tag="gq")
            gk0 = pool_g.tile([P, RB, SC], BF16, tag="gk")
            gv0 = pool_g.tile([P, RB, SC], BF16, tag="gv")
            # fp8 qk stream first so the first DR matmuls start after ~2
            # blocks (subtile deps); per-rb gate pieces behind them; the bf16
            # x / v_read stream (A-v inputs) after, then the small B weights
            for db in range(DB):
                nc.sync.dma_start(qr8_sb[:, db, :], _r(qk_readT8)[:, db, :])
                nc.sync.dma_start(x80[:, db, :], _r(xT8)[:, db, 0:SC])
                if db >= 4 and db < 8:
                    rb = db - 4
                    nc.sync.dma_start(gq0[:, rb, :], _r(gqT)[:, rb, 0:SC])
                    nc.sync.dma_start(gk0[:, rb, :], _r(gkT)[:, rb, 0:SC])
                    nc.sync.dma_start(gv0[:, rb, :], _r(gvT)[:, rb, 0:SC])
            nc.sync.dma_start(qw8_sb[:], _r(qk_w))
            for db in range(DB):
                nc.sync.dma_start(vr_sb[:, db, :], _r(v_readT)[:, db, :])
                nc.sync.dma_start(xt0[:, db, :], _r(xT)[:, db, 0:SC])
            nc.sync.dma_start(vw_sb[:], _r(v_w))
            for sc_i in range(NSC):
                sl = slice(sc_i * SC, (sc_i + 1) * SC)
                if sc_i == 0:
                    xt, x8, gq, gk, gv = xt0, x80, gq0, gk0, gv0
                else:
                    x8 = pool_x.tile([P, DB, SC], F8, tag="x8")
                    nc.sync.dma_start(x8[:], _r(xT8)[:, :, sl])
                    xt = pool_x.tile([P, DB, SC], BF16, tag="xt")
                    nc.sync.dma_start(xt[:], _r(xT)[:, :, sl])
                    gq = pool_g.tile([P, RB, SC], BF16, tag="gq")
                    nc.sync.dma_start(gq[:], _r(gqT)[:, :, sl])
                    gk = pool_g.tile([P, RB, SC], BF16, tag="gk")
                    nc.sync.dma_start(gk[:], _r(gkT)[:, :, sl])
                    gv = pool_g.tile([P, RB, SC], BF16, tag="gv")
                    nc.sync.dma_start(gv[:], _r(gvT)[:, :, sl])
                qg = pool_gch.tile([P, RB, SC], F8, tag="qg")
                kg = pool_gch.tile([P, RB, SC], F8, tag="kg")
                vg = pool_gch.tile([P, RB, SC], BF16, tag="vg")
                for rb in range(RB):
                    ps = psA.tile([P, SC], F32, tag="tA")
                    # fp8 DoubleRow: two d-blocks per matmul, half the PE time
                    for dp in range(DB // 2):
                        nc.tensor.matmul(ps[:],
                                         qr8_sb[:, 2 * dp:2 * dp + 2,
                                                rb * P:(rb + 1) * P],
                                         x8[:, 2 * dp:2 * dp + 2, :],
                                         start=(dp == 0),
                                         stop=(dp == DB // 2 - 1),
                                         perf_mode=DR)
                    nc.vector.tensor_tensor(qg[:, rb, :], ps[:], gq[:, rb, :],
                                            ALU.mult)
                    nc.vector.tensor_tensor(kg[:, rb, :], ps[:], gk[:, rb, :],
                                            ALU.mult)
                for rb in range(RB):
                    ps = psA.tile([P, SC], F32, tag="tA")
                    for db in range(DB):
                        nc.tensor.matmul(ps[:], vr_sb[:, db, rb * P:(rb + 1) * P],
                                         xt[:, db, :], start=(db == 0),
                                         stop=(db == DB - 1))
                    nc.vector.tensor_tensor(vg[:, rb, :], ps[:], gv[:, rb, :],
                                            ALU.mult)
                # B1: Q^T / K^T [d', s] per 128-col block of DHG (fp8 DR)
                for dbk in range(HG):
                    dsl = slice(dbk * P, (dbk + 1) * P)
                    psq = psB.tile([P, SC], F32, tag="qB")
                    for rp in range(RB // 2):
                        nc.tensor.matmul(psq[:],
                                         qw8_sb[:, 2 * rp:2 * rp + 2, dsl],
                                         qg[:, 2 * rp:2 * rp + 2, :],
                                         start=(rp == 0),
                                         stop=(rp == RB // 2 - 1),
                                         perf_mode=DR)
                    nc.scalar.activation(QT_sb[:, dbk, sl], psq[:], AF.Copy)
                    psk = psB.tile([P, SC], F32, tag="kB")
                    for rp in range(RB // 2):
                        nc.tensor.matmul(psk[:],
                                         qw8_sb[:, 2 * rp:2 * rp + 2, dsl],
                                         kg[:, 2 * rp:2 * rp + 2, :],
                                         start=(rp == 0),
                                         stop=(rp == RB // 2 - 1),
                                         perf_mode=DR)
                    nc.scalar.activation(KT_sb[:, dbk, sl], psk[:], AF.Copy)
                # B2: V natural [s, d']
                for sj in range(SC // P):
                    s_blk = sc_i * (SC // P) + sj
                    psv = psB.tile([P, DHG], F32, tag="vB")
                    for rb in range(RB):
                        nc.tensor.matmul(psv[:], vg[:, rb, sj * P:(sj + 1) * P],
                                         vw_sb[:, rb, :], start=(rb == 0),
                                         stop=(rb == RB - 1))
                    nc.scalar.activation(V_sb[:, s_blk, :], psv[:], AF.Copy)

        # ========== C + D: attention, AllGather, W_O ==========
        with (
            tc.tile_pool(name="csmall", bufs=1) as pool_c1,
            tc.tile_pool(name="exp", bufs=3) as pool_exp,
            tc.tile_pool(name="recip", bufs=2) as pool_recip,
            tc.tile_pool(name="ao", bufs=2) as pool_ao,
            tc.tile_pool(name="wo", bufs=1) as pool_wo,
            tc.tile_pool(name="aof", bufs=4) as pool_aof,
            tc.tile_pool(name="osb", bufs=2) as pool_osb,
            tc.tile_pool(name="dram0", bufs=1, space="DRAM") as pool_dram0,
            tc.tile_pool(name="dram1", bufs=1, space="DRAM") as pool_dram1,
            tc.tile_pool(name="dram2", bufs=1, space="DRAM") as pool_dram2,
            tc.tile_pool(name="dram3", bufs=1, space="DRAM") as pool_dram3,
            tc.tile_pool(name="psS", bufs=2, space="PSUM") as psS,
            tc.tile_pool(name="psPV", bufs=1, space="PSUM") as psPV,
            tc.tile_pool(name="psRS", bufs=1, space="PSUM") as psRS,
            tc.tile_pool(name="psD", bufs=2, space="PSUM") as psD,
        ):
            pool_drams = [pool_dram0, pool_dram1, pool_dram2, pool_dram3]
            # constants
            masks_f = pool_c1.tile([P, HG, 512], F32)
            nc.vector.memset(masks_f[:], 1.0)
            for o in range(HG):
                nc.gpsimd.affine_select(
                    out=masks_f[:, o, :], in_=masks_f[:, o, :],
                    compare_op=ALU.is_ge, fill=0.0, base=-P * o,
                    pattern=[[1, 512]], channel_multiplier=-1)
            masks = pool_c1.tile([P, HG, 512], BF16)
            nc.vector.tensor_copy(masks[:], masks_f[:])
            ones128 = pool_c1.tile([P, P], BF16)
            nc.vector.memset(ones128[:], 0.81)  # 1/KEEP^2; residual in wo
            wo_sb = pool_wo.tile([P, DB, DHG], BF16)
            for db in range(DB):
                nc.sync.dma_start(wo_sb[:, db, :], _r(wo)[:, db, :])

            def head_tail(ao, h, pv, rsum):
                """ao[:,h,:] = pv / rsum (rowsum already replicated)."""
                recip = pool_recip.tile([P, 512], F32, tag="recip")
                nc.vector.reciprocal_approx_fast(recip[:], rsum[:])
                nc.vector.tensor_tensor(ao[:, h, :], pv[:], recip[:], ALU.mult)

            ag_outs = {}

            aof_tiles = [None] * NT

            def issue_aof(t):
                """prefetch the gathered AO chunk as soon as AG(t) is done;
                two half DMAs so D can start on the first half earlier"""
                aof = pool_aof.tile([P, DB, 512], BF16, tag="aof", bufs=2)
                nc.sync.dma_start(aof[:, :, 0:256], _r(ag_outs[t])[:, :, 0:256])
                nc.sync.dma_start(aof[:, :, 256:512],
                                  _r(ag_outs[t])[:, :, 256:512])
                aof_tiles[t] = aof

            SPLIT_AG = (0, NT - 1)
            aofh = {}

            def emit_half_ag(t, half, ao_slice):
                bin_h = pool_drams[t].tile([2 * P, 512], BF16,
                                           tag=f"bin_{half}",
                                           name=f"bin{t}{half}")
                bout_h = pool_drams[t].tile([8 * P, 512], BF16,
                                            tag=f"bout_{half}",
                                            name=f"bout{t}{half}")
                nc.sync.dma_start(
                    bin_h.rearrange("(h p) s -> p h s", p=P), ao_slice)
                nc.gpsimd.collective_compute(
                    "AllGather", ALU.bypass, ins=[bin_h[:].opt()],
                    outs=[bout_h[:].opt()], replica_groups=RGROUPS)
                aofh[(t, half)] = pool_aof.tile(
                    [P, 8, 512], BF16, tag=f"aof{t}{half}",
                    name=f"aof{t}{half}", bufs=1)
                nc.sync.dma_start(aofh[(t, half)][:], _r(bout_h))

            def emit_D(t):
                for si in range(4):
                    ssl = slice(si * P, (si + 1) * P)
                    ps = psD.tile([P, DHG], F32, tag="d")
                    if t in SPLIT_AG:
                        # halves: heads 0-1 arrived via the early half-AG
                        for half, hoff in ((aofh[(t, 'a')], 0),
                                           (aofh[(t, 'b')], 2)):
                            for o2 in range(8):
                                dbk = (o2 // 2) * 4 + hoff + (o2 % 2)
                                nc.tensor.matmul(
                                    ps[:], half[:, o2, ssl], wo_sb[:, dbk, :],
                                    start=(hoff == 0 and o2 == 0),
                                    stop=(hoff == 2 and o2 == 7))
                    else:
                        aof = aof_tiles[t]
                        for dbk in range(DB):
                            nc.tensor.matmul(ps[:], aof[:, dbk, ssl],
                                             wo_sb[:, dbk, :],
                                             start=(dbk == 0),
                                             stop=(dbk == DB - 1))
                    o_sb = pool_osb.tile([P, DHG], F32, tag="osb")
                    nc.scalar.activation(o_sb[:], ps[:], AF.Copy)
                    row0 = (t * 4 + si) * P
                    nc.sync.dma_start(out[row0:row0 + P, :], o_sb[:])

            for t in range(NT):
                tsl = slice(t * 512, (t + 1) * 512)
                ao = pool_ao.tile([P, HG, 512], BF16, tag="ao")
                npair = 2 * (t + 1)
                prev = None
                for h in range(HG):
                    pv = psPV.tile([P, 512], F32, tag="pv")
                    rsum = psRS.tile([P, 512], F32, tag="rs")
                    for p in range(npair):
                        # paired key blocks j=2p,2p+1: 2 score matmuls into one
                        # 2-bank PSUM tile, a single exp over 1024 cols keeps
                        # ScalarE off the per-block critical path
                        scp = psS.tile([P, 2, 512], F32, tag="sc")
                        for i in range(2):
                            j = 2 * p + i
                            nc.tensor.matmul(scp[:, i, :],
                                             KT_sb[:, h, j * P:(j + 1) * P],
                                             QT_sb[:, h, tsl],
                                             start=True, stop=True)
                        etp = pool_exp.tile([P, 2, 512], BF16, tag="et")
                        nc.scalar.activation(etp[:], scp[:], AF.Exp,
                                             scale=EXP_SCALE)
                        dp = p - 2 * t
                        if dp >= 0:
                            nc.vector.tensor_tensor(
                                etp[:], etp[:],
                                masks[:, 2 * dp:2 * dp + 2, :], ALU.mult)
                        for i in range(2):
                            j = 2 * p + i
                            nc.tensor.matmul(rsum[:], ones128[:],
                                             etp[:, i, :], start=(j == 0),
                                             stop=(j == 2 * npair - 1))
                            nc.tensor.matmul(pv[:],
                                             V_sb[:, j, h * P:(h + 1) * P],
                                             etp[:, i, :], start=(j == 0),
                                             stop=(j == 2 * npair - 1))
                        if p == 0 and prev is not None:
                            head_tail(ao, *prev)   # overlap prior head's tail
                            prev = None
                            if t in SPLIT_AG and h == 2:
                                # heads 0-1 done: gather them now (smaller
                                # rendezvous, earlier start than a full AG)
                                emit_half_ag(t, 'a', ao[:, 0:2, :])
                    prev = (h, pv, rsum)
                head_tail(ao, *prev)
                # AllGather this si-chunk across the 4-core group (bf16).
                # bin/bout live in per-chunk DRAM pools so chunk t+1's staging
                # DMA is not serialized behind collective t.
                if t in SPLIT_AG:
                    emit_half_ag(t, 'b', ao[:, 2:4, :])
                else:
                    bin_t = pool_drams[t].tile([DHG, 512], BF16, tag="bin")
                    bout_t = pool_drams[t].tile([D, 512], BF16, tag="bout")
                    nc.sync.dma_start(
                        bin_t.rearrange("(h p) s -> p h s", p=P), ao[:])
                    nc.gpsimd.collective_compute(
                        "AllGather", ALU.bypass, ins=[bin_t[:].opt()],
                        outs=[bout_t[:].opt()], replica_groups=RGROUPS)
                    ag_outs[t] = bout_t
                if t == 2:
                    issue_aof(1)
            issue_aof(2)
            # soft sim-time floors, dialed so each D(t) lands just after its
            # aof(t) is ready in real time: late enough not to stall on the
            # AllGather, early enough to fill PE during later collectives (a
            # too-late pin serializes D behind every earlier DMA via
            # conservative queue-counter waits)
            for t, pin in zip(range(NT), (0.112, 0.125, 0.14, 0.165)):
                with tc.tile_wait_until(pin):
                    emit_D(t)


def _get_nc():
    if 'nc' not in _CACHE:
        _CACHE['nc'] = _build()
    return _CACHE['nc']


def _bf(a):
    return np.ascontiguousarray(a).astype(ml_dtypes.bfloat16)


def kernel(**inputs):
    x = np.asarray(inputs["x"], np.float32)
    g_Q = np.asarray(inputs["g_Q"], np.float32)
    g_K = np.asarray(inputs["g_K"], np.float32)
    g_V = np.asarray(inputs["g_V"], np.float32)
    qk_read = np.asarray(inputs["qk_read"], np.float32)
    qk_write = np.asarray(inputs["qk_write"], np.float32)
    v_read = np.asarray(inputs["v_read"], np.float32)
    v_write = np.asarray(inputs["v_write"], np.float32)
    W_O = np.asarray(inputs["W_O"], np.float32)

    nc = _get_nc()
    qk_readT8 = np.ascontiguousarray(qk_read.T * QK_PRESCALE).astype(
        ml_dtypes.float8_e4m3)
    v_readT = _bf(v_read.T)
    # the in-kernel rowsum scale is bf16(0.81); fold the exact residual
    # (and the intended 1/0.81 dropout scaling) into W_O host-side
    wo_fix = W_O * (KEEP2_BF / 0.81)
    in_maps = []
    for c in range(8):
        b, hg = divmod(c, 4)
        cs = slice(hg * DHG, (hg + 1) * DHG)
        xbT = np.ascontiguousarray(x[b].T)
        in_maps.append({
            "xT": xbT.astype(ml_dtypes.bfloat16),
            "xT8": xbT.astype(ml_dtypes.float8_e4m3),
            "gqT": _bf(g_Q[b].T),
            "gkT": _bf(g_K[b].T),
            "gvT": _bf(g_V[b].T),
            "qk_readT8": qk_readT8,
            "v_readT": v_readT,
            "qk_write_hg": np.ascontiguousarray(
                qk_write[:, cs] * QK_PRESCALE).astype(ml_dtypes.float8_e4m3),
            "v_write_hg": _bf(v_write[:, cs]),
            "wo_cols": _bf(wo_fix[:, cs]),
        })
    res = run_bass_kernel_spmd(nc, in_maps, core_ids=list(range(8)))
    _CACHE['last_results'] = res
    out = np.empty((B, S, D), np.float32)
    for c in range(8):
        b, hg = divmod(c, 4)
        out[b, :, hg * DHG:(hg + 1) * DHG] = res.results[c]["out"]
    return out


# revision 67
# speedup vs baseline: 1.3967x; 1.3967x over previous
"""Distributed Bass kernel for nn_AttentionCircuit (B=2,S=2048,D=2048,RANK=512,H=16).

Sharding: 8 cores = 2 batches x 4 head-groups (4 heads / 512 D-cols each).
All matmuls in bf16 (1 cycle/row on PE, half the DMA/SBUF of fp32).

Per-core dataflow (contraction always on the partition axis, no on-device
transposes; host pre-transposes x / gates):
  AB (fused, one pass over x^T, s-chunks of 512):
     t_qk^T = qk_read @ x^T ; gate -> Qg^T, Kg^T   (bf16)
     t_v^T  = v_read  @ x^T ; gate -> Vg^T
     Q^T/K^T = qk_write_hg.T @ {Q,K}g^T   (transposed [d',s], bf16 SBUF)
     V       = Vg^T.T @ v_write_hg        (natural [s,d'], bf16 SBUF)
  C: per 512-query chunk t, per head: scores^T = K^T.T Q^T -> exp (bf16, no
     max-sub; logits are tiny) -> causal mask (block-skip + diag masks) ->
     replicated rowsum via ones128-matmul -> PV matmul -> ao = pv * recip
     -> AllGather(group of 4, bf16) overlapped with next chunks
  D: out_cols = AO_full^T.T @ W_O[:,cols], chunks interleaved after AG(t)
     completes so the readback DMA spreads across phase C.
"""
import sys
import numpy as np
import ml_dtypes

sys.path.insert(0, '/opt/trn_rl_repo')

import concourse.bass as bass  # noqa: E402
from concourse import bacc  # noqa: E402
import concourse.mybir as mybir  # noqa: E402
import concourse.tile as tile  # noqa: E402
from concourse.bass_utils import run_bass_kernel_spmd  # noqa: E402

B, S, D = 2, 2048, 2048
RANK = 512
NH = 16
HG = 4              # head-groups == cores per batch
DHG = D // HG       # 512 cols per head-group (4 heads)
DH = D // NH        # 128 head dim
P = 128
DB = D // P         # 16 d-blocks
RB = RANK // P      # 4 rank-blocks
SB = S // P         # 16 s-blocks
NT = S // 512       # 4 si tiles of 512
SC = 512            # fused-A s-chunk width
NSC = S // SC       # 4

F32 = mybir.dt.float32
BF16 = mybir.dt.bfloat16
F8 = mybir.dt.float8e4
DR = mybir.MatmulPerfMode.DoubleRow
AF = mybir.ActivationFunctionType
ALU = mybir.AluOpType

QK_PRESCALE = 16.0  # host scales qk_read AND qk_write by 16 into fp8 range
EXP_SCALE = 1.0 / float(np.sqrt(DH)) / QK_PRESCALE ** 4
KEEP2_Q = 0.8125    # fp8e4m3(0.81); exact residual folded into wo host-side
KEEP2_BF = float(np.float32(ml_dtypes.bfloat16(0.81)))  # ones128 constant as hw sees it
RGROUPS = [[0, 1, 2, 3], [4, 5, 6, 7]]

_CACHE = {}


def _r(ap):
    """[ (o p), f ] DRAM tensor -> [p, o, f] partition-tiled view."""
    return ap.rearrange("(o p) f -> p o f", p=P)


def _build():
    nc = bacc.Bacc("TRN2", target_bir_lowering=False, debug=False,
                   enable_asserts=False, num_devices=8)
    xT = nc.dram_tensor("xT", [D, S], BF16, kind="ExternalInput").ap()
    xT8 = nc.dram_tensor("xT8", [D, S], F8, kind="ExternalInput").ap()
    gqT = nc.dram_tensor("gqT", [RANK, S], BF16, kind="ExternalInput").ap()
    gkT = nc.dram_tensor("gkT", [RANK, S], BF16, kind="ExternalInput").ap()
    gvT = nc.dram_tensor("gvT", [RANK, S], BF16, kind="ExternalInput").ap()
    qk_readT8 = nc.dram_tensor("qk_readT8", [D, RANK], F8, kind="ExternalInput").ap()
    v_readT = nc.dram_tensor("v_readT", [D, RANK], BF16, kind="ExternalInput").ap()
    qk_w = nc.dram_tensor("qk_write_hg", [RANK, DHG], F8, kind="ExternalInput").ap()
    v_w = nc.dram_tensor("v_write_hg", [RANK, DHG], BF16, kind="ExternalInput").ap()
    wo = nc.dram_tensor("wo_cols", [D, DHG], BF16, kind="ExternalInput").ap()
    out = nc.dram_tensor("out", [S, DHG], F32, kind="ExternalOutput").ap()

    with tile.TileContext(nc) as tc:
        _body(tc, xT, xT8, gqT, gkT, gvT, qk_readT8, v_readT, qk_w, v_w, wo,
              out)
    nc.compile()
    return nc


def _body(tc, xT, xT8, gqT, gkT, gvT, qk_readT8, v_readT, qk_w, v_w, wo, out):
    nc = tc.nc
    import contextlib
    ctx = contextlib.ExitStack()
    with ctx:
        # ---- long-lived activation tensors
        pool_qk = ctx.enter_context(tc.tile_pool(name="qk", bufs=1))
        QT_sb = pool_qk.tile([P, HG, S], BF16)          # Q^T [d', s]
        KT_sb = pool_qk.tile([P, HG, S], BF16)
        V_sb = pool_qk.tile([P, SB, DHG], BF16)         # V natural [s, d']

        # warmup AllGather at sim-time 0: pays the first-collective setup
        # cost during phase AB instead of in front of chunk 0's AG; sized
        # 128KB so the data path (not just control) is warmed
        pool_warm = ctx.enter_context(tc.tile_pool(name="warm", bufs=1))
        with tc.high_priority():
            warm_sb = pool_warm.tile([P, 512], BF16)
            nc.vector.memset(warm_sb[:], 0.0)
            warm_in = pool_warm.tile([P, 512], BF16, space="DRAM")
            warm_out = pool_warm.tile([4 * P, 512], BF16, space="DRAM")
            nc.sync.dma_start(warm_in[:], warm_sb[:])
            nc.gpsimd.collective_compute(
                "AllGather", ALU.bypass, ins=[warm_in[:].opt()],
                outs=[warm_out[:].opt()], replica_groups=RGROUPS)
        # second tiny warmup mid-AB: acts as a cross-core barrier so peers
        # reach chunk 0's AllGather with less skew (its data phase is mostly
        # rendezvous wait, not bandwidth)
        warm2_in = pool_warm.tile([P, 1], BF16, space="DRAM")
        warm2_out = pool_warm.tile([4 * P, 1], BF16, space="DRAM")
        with tc.tile_wait_until(0.08):
            nc.sync.dma_start(warm2_in[:], warm_sb[:, 0:1])
            nc.gpsimd.collective_compute(
                "AllGather", ALU.bypass, ins=[warm2_in[:].opt()],
                outs=[warm2_out[:].opt()], replica_groups=RGROUPS)

        # ========== fused A+B per s-chunk: Q^T, K^T, V ==========
        with (
            tc.tile_pool(name="rd", bufs=1) as pool_rd,
            tc.tile_pool(name="wr", bufs=1) as pool_wr,
            tc.tile_pool(name="ax", bufs=2) as pool_x,
            tc.tile_pool(name="ag", bufs=2) as pool_g,
            tc.tile_pool(name="gch", bufs=2) as pool_gch,
            tc.tile_pool(name="psA", bufs=2, space="PSUM") as psA,
            tc.tile_pool(name="psB", bufs=2, space="PSUM") as psB,
        ):
            qr8_sb = pool_rd.tile([P, DB, RANK], F8)
            vr_sb = pool_rd.tile([P, DB, RANK], BF16)
            qw8_sb = pool_wr.tile([P, RB, DHG], F8)
            vw_sb = pool_wr.tile([P, RB, DHG], BF16)
            xt0 = pool_x.tile([P, DB, SC], BF16, tag="xt")
            x80 = pool_x.tile([P, DB, SC], F8, tag="x8")
            gq0 = pool_g.tile([P, RB, SC], BF16, tag="gq")
            gk0 = pool_g.tile([P, RB, SC], BF16, tag="gk")
            gv0 = pool_g.tile([P, RB, SC], BF16, tag="gv")
            # fp8 qk stream first so the first DR matmuls start after ~2
            # blocks (subtile deps); per-rb gate pieces behind them; the bf16
            # x / v_read stream (A-v inputs) after, then the small B weights
            for db in range(DB):
                nc.sync.dma_start(qr8_sb[:, db, :], _r(qk_readT8)[:, db, :])
                nc.sync.dma_start(x80[:, db, :], _r(xT8)[:, db, 0:SC])
                if db >= 4 and db < 8:
                    rb = db - 4
                    nc.sync.dma_start(gq0[:, rb, :], _r(gqT)[:, rb, 0:SC])
                    nc.sync.dma_start(gk0[:, rb, :], _r(gkT)[:, rb, 0:SC])
                    nc.sync.dma_start(gv0[:, rb, :], _r(gvT)[:, rb, 0:SC])
            nc.sync.dma_start(qw8_sb[:], _r(qk_w))
            for db in range(DB):
                nc.sync.dma_start(vr_sb[:, db, :], _r(v_readT)[:, db, :])
                nc.sync.dma_start(xt0[:, db, :], _r(xT)[:, db, 0:SC])
            nc.sync.dma_start(vw_sb[:], _r(v_w))
            for sc_i in range(NSC):
                sl = slice(sc_i * SC, (sc_i + 1) * SC)
                if sc_i == 0:
                    xt, x8, gq, gk, gv = xt0, x80, gq0, gk0, gv0
                else:
                    x8 = pool_x.tile([P, DB, SC], F8, tag="x8")
                    nc.sync.dma_start(x8[:], _r(xT8)[:, :, sl])
                    xt = pool_x.tile([P, DB, SC], BF16, tag="xt")
                    nc.sync.dma_start(xt[:], _r(xT)[:, :, sl])
                    gq = pool_g.tile([P, RB, SC], BF16, tag="gq")
                    nc.sync.dma_start(gq[:], _r(gqT)[:, :, sl])
                    gk = pool_g.tile([P, RB, SC], BF16, tag="gk")
                    nc.sync.dma_start(gk[:], _r(gkT)[:, :, sl])
                    gv = pool_g.tile([P, RB, SC], BF16, tag="gv")
                    nc.sync.dma_start(gv[:], _r(gvT)[:, :, sl])
                qg = pool_gch.tile([P, RB, SC], F8, tag="qg")
                kg = pool_gch.tile([P, RB, SC], F8, tag="kg")
                vg = pool_gch.tile([P, RB, SC], BF16, tag="vg")
                for rb in range(RB):
                    ps = psA.tile([P, SC], F32, tag="tA")
                    # fp8 DoubleRow: two d-blocks per matmul, half the PE time
                    for dp in range(DB // 2):
                        nc.tensor.matmul(ps[:],
                                         qr8_sb[:, 2 * dp:2 * dp + 2,
                                                rb * P:(rb + 1) * P],
                                         x8[:, 2 * dp:2 * dp + 2, :],
                                         start=(dp == 0),
                                         stop=(dp == DB // 2 - 1),
                                         perf_mode=DR)
                    nc.vector.tensor_tensor(qg[:, rb, :], ps[:], gq[:, rb, :],
                                            ALU.mult)
                    nc.vector.tensor_tensor(kg[:, rb, :], ps[:], gk[:, rb, :],
                                            ALU.mult)
                for rb in range(RB):
                    ps = psA.tile([P, SC], F32, tag="tA")
                    for db in range(DB):
                        nc.tensor.matmul(ps[:], vr_sb[:, db, rb * P:(rb + 1) * P],
                                         xt[:, db, :], start=(db == 0),
                                         stop=(db == DB - 1))
                    nc.vector.tensor_tensor(vg[:, rb, :], ps[:], gv[:, rb, :],
                                            ALU.mult)
                # B1: Q^T / K^T [d', s] per 128-col block of DHG (fp8 DR)
                for dbk in range(HG):
                    dsl = slice(dbk * P, (dbk + 1) * P)
                    psq = psB.tile([P, SC], F32, tag="qB")
                    for rp in range(RB // 2):
                        nc.tensor.matmul(psq[:],
                                         qw8_sb[:, 2 * rp:2 * rp + 2, dsl],
                                         qg[:, 2 * rp:2 * rp + 2, :],
                                         start=(rp == 0),
                                         stop=(rp == RB // 2 - 1),
                                         perf_mode=DR)
                    nc.scalar.activation(QT_sb[:, dbk, sl], psq[:], AF.Copy)
                    psk = psB.tile([P, SC], F32, tag="kB")
                    for rp in range(RB // 2):
                        nc.tensor.matmul(psk[:],
                                         qw8_sb[:, 2 * rp:2 * rp + 2, dsl],
                                         kg[:, 2 * rp:2 * rp + 2, :],
                                         start=(rp == 0),
                                         stop=(rp == RB // 2 - 1),
                                         perf_mode=DR)
                    nc.scalar.activation(KT_sb[:, dbk, sl], psk[:], AF.Copy)
                # B2: V natural [s, d']
                for sj in range(SC // P):
                    s_blk = sc_i * (SC // P) + sj
                    psv = psB.tile([P, DHG], F32, tag="vB")
                    for rb in range(RB):
                        nc.tensor.matmul(psv[:], vg[:, rb, sj * P:(sj + 1) * P],
                                         vw_sb[:, rb, :], start=(rb == 0),
                                         stop=(rb == RB - 1))
                    nc.scalar.activation(V_sb[:, s_blk, :], psv[:], AF.Copy)

        # ========== C + D: attention, AllGather, W_O ==========
        with (
            tc.tile_pool(name="csmall", bufs=1) as pool_c1,
            tc.tile_pool(name="exp", bufs=3) as pool_exp,
            tc.tile_pool(name="recip", bufs=2) as pool_recip,
            tc.tile_pool(name="ao", bufs=2) as pool_ao,
            tc.tile_pool(name="wo", bufs=1) as pool_wo,
            tc.tile_pool(name="aof", bufs=4) as pool_aof,
            tc.tile_pool(name="osb", bufs=2) as pool_osb,
            tc.tile_pool(name="dram0", bufs=1, space="DRAM") as pool_dram0,
            tc.tile_pool(name="dram1", bufs=1, space="DRAM") as pool_dram1,
            tc.tile_pool(name="dram2", bufs=1, space="DRAM") as pool_dram2,
            tc.tile_pool(name="dram3", bufs=1, space="DRAM") as pool_dram3,
            tc.tile_pool(name="psS", bufs=2, space="PSUM") as psS,
            tc.tile_pool(name="psPV", bufs=1, space="PSUM") as psPV,
            tc.tile_pool(name="psRS", bufs=1, space="PSUM") as psRS,
            tc.tile_pool(name="psD", bufs=2, space="PSUM") as psD,
        ):
            pool_drams = [pool_dram0, pool_dram1, pool_dram2, pool_dram3]
            # constants
            masks_f = pool_c1.tile([P, HG, 512], F32)
            nc.vector.memset(masks_f[:], 1.0)
            for o in range(HG):
                nc.gpsimd.affine_select(
                    out=masks_f[:, o, :], in_=masks_f[:, o, :],
                    compare_op=ALU.is_ge, fill=0.0, base=-P * o,
                    pattern=[[1, 512]], channel_multiplier=-1)
            masks = pool_c1.tile([P, HG, 512], BF16)
            nc.vector.tensor_copy(masks[:], masks_f[:])
            ones128 = pool_c1.tile([P, P], BF16)
            nc.vector.memset(ones128[:], 0.81)  # 1/KEEP^2; residual in wo
            wo_sb = pool_wo.tile([P, DB, DHG], BF16)
            for db in range(DB):
                nc.sync.dma_start(wo_sb[:, db, :], _r(wo)[:, db, :])

            def head_tail(ao, h, pv, rsum):
                """ao[:,h,:] = pv / rsum (rowsum already replicated)."""
                recip = pool_recip.tile([P, 512], F32, tag="recip")
                nc.vector.reciprocal_approx_fast(recip[:], rsum[:])
                nc.vector.tensor_tensor(ao[:, h, :], pv[:], recip[:], ALU.mult)

            ag_outs = {}

            aof_tiles = [None] * NT

            def issue_aof(t):
                """prefetch the gathered AO chunk as soon as AG(t) is done;
                two half DMAs so D can start on the first half earlier"""
                aof = pool_aof.tile([P, DB, 512], BF16, tag="aof", bufs=2)
                nc.sync.dma_start(aof[:, :, 0:256], _r(ag_outs[t])[:, :, 0:256])
                nc.sync.dma_start(aof[:, :, 256:512],
                                  _r(ag_outs[t])[:, :, 256:512])
                aof_tiles[t] = aof

            SPLIT_AG = (0, NT - 1)
            aofh = {}

            def emit_half_ag(t, half, ao_slice):
                bin_h = pool_drams[t].tile([2 * P, 512], BF16,
                                           tag=f"bin_{half}",
                                           name=f"bin{t}{half}")
                bout_h = pool_drams[t].tile([8 * P, 512], BF16,
                                            tag=f"bout_{half}",
                                            name=f"bout{t}{half}")
                nc.sync.dma_start(
                    bin_h.rearrange("(h p) s -> p h s", p=P), ao_slice)
                nc.gpsimd.collective_compute(
                    "AllGather", ALU.bypass, ins=[bin_h[:].opt()],
                    outs=[bout_h[:].opt()], replica_groups=RGROUPS)
                aofh[(t, half)] = pool_aof.tile(
                    [P, 8, 512], BF16, tag=f"aof{t}{half}",
                    name=f"aof{t}{half}", bufs=1)
                nc.sync.dma_start(aofh[(t, half)][:], _r(bout_h))

            def emit_D(t):
                for si in range(4):
                    ssl = slice(si * P, (si + 1) * P)
                    ps = psD.tile([P, DHG], F32, tag="d")
                    if t in SPLIT_AG:
                        # halves: heads 0-1 arrived via the early half-AG
                        for half, hoff in ((aofh[(t, 'a')], 0),
                                           (aofh[(t, 'b')], 2)):
                            for o2 in range(8):
                                dbk = (o2 // 2) * 4 + hoff + (o2 % 2)
                                nc.tensor.matmul(
                                    ps[:], half[:, o2, ssl], wo_sb[:, dbk, :],
                                    start=(hoff == 0 and o2 == 0),
                                    stop=(hoff == 2 and o2 == 7))
                    else:
                        aof = aof_tiles[t]
                        for dbk in range(DB):
                            nc.tensor.matmul(ps[:], aof[:, dbk, ssl],
                                             wo_sb[:, dbk, :],
                                             start=(dbk == 0),
                                             stop=(dbk == DB - 1))
                    o_sb = pool_osb.tile([P, DHG], F32, tag="osb")
                    nc.scalar.activation(o_sb[:], ps[:], AF.Copy)
                    row0 = (t * 4 + si) * P
                    nc.sync.dma_start(out[row0:row0 + P, :], o_sb[:])

            for t in range(NT):
                tsl = slice(t * 512, (t + 1) * 512)
                ao = pool_ao.tile([P, HG, 512], BF16, tag="ao")
                npair = 2 * (t + 1)
                prev = None
                for h in range(HG):
                    pv = psPV.tile([P, 512], F32, tag="pv")
                    rsum = psRS.tile([P, 512], F32, tag="rs")
                    for p in range(npair):
                        # paired key blocks j=2p,2p+1: 2 score matmuls into one
                        # 2-bank PSUM tile, a single exp over 1024 cols keeps
                        # ScalarE off the per-block critical path
                        scp = psS.tile([P, 2, 512], F32, tag="sc")
                        for i in range(2):
                            j = 2 * p + i
                            nc.tensor.matmul(scp[:, i, :],
                                             KT_sb[:, h, j * P:(j + 1) * P],
                                             QT_sb[:, h, tsl],
                                             start=True, stop=True)
                        etp = pool_exp.tile([P, 2, 512], BF16, tag="et")
                        nc.scalar.activation(etp[:], scp[:], AF.Exp,
                                             scale=EXP_SCALE)
                        dp = p - 2 * t
                        if dp >= 0:
                            nc.vector.tensor_tensor(
                                etp[:], etp[:],
                                masks[:, 2 * dp:2 * dp + 2, :], ALU.mult)
                        for i in range(2):
                            j = 2 * p + i
                            nc.tensor.matmul(rsum[:], ones128[:],
                                             etp[:, i, :], start=(j == 0),
                                             stop=(j == 2 * npair - 1))
                            nc.tensor.matmul(pv[:],
                                             V_sb[:, j, h * P:(h + 1) * P],
                                             etp[:, i, :], start=(j == 0),
                                             stop=(j == 2 * npair - 1))
                        if p == 0 and prev is not None:
                            head_tail(ao, *prev)   # overlap prior head's tail
                            prev = None
                            if t in SPLIT_AG and h == 2:
                                # heads 0-1 done: gather them now (smaller
                                # rendezvous, earlier start than a full AG)
                                emit_half_ag(t, 'a', ao[:, 0:2, :])
                    prev = (h, pv, rsum)
                head_tail(ao, *prev)
                # AllGather this si-chunk across the 4-core group (bf16).
                # bin/bout live in per-chunk DRAM pools so chunk t+1's staging
                # DMA is not serialized behind collective t.
                if t in SPLIT_AG:
                    emit_half_ag(t, 'b', ao[:, 2:4, :])
                else:
                    bin_t = pool_drams[t].tile([DHG, 512], BF16, tag="bin")
                    bout_t = pool_drams[t].tile([D, 512], BF16, tag="bout")
                    nc.sync.dma_start(
                        bin_t.rearrange("(h p) s -> p h s", p=P), ao[:])
                    nc.gpsimd.collective_compute(
                        "AllGather", ALU.bypass, ins=[bin_t[:].opt()],
                        outs=[bout_t[:].opt()], replica_groups=RGROUPS)
                    ag_outs[t] = bout_t
                if t == 2:
                    issue_aof(1)
            issue_aof(2)
            # soft sim-time floors, dialed so each D(t) lands just after its
            # aof(t) is ready in real time: late enough not to stall on the
            # AllGather, early enough to fill PE during later collectives (a
            # too-late pin serializes D behind every earlier DMA via
            # conservative queue-counter waits)
            for t, pin in zip(range(NT), (0.112, 0.125, 0.14, 0.165)):
                with tc.tile_wait_until(pin):
                    emit_D(t)


def _get_nc():
    if 'nc' not in _CACHE:
        _CACHE['nc'] = _build()
    return _CACHE['nc']


def _bf(a):
    return np.ascontiguousarray(a).astype(ml_dtypes.bfloat16)


def kernel(**inputs):
    x = np.asarray(inputs["x"], np.float32)
    g_Q = np.asarray(inputs["g_Q"], np.float32)
    g_K = np.asarray(inputs["g_K"], np.float32)
    g_V = np.asarray(inputs["g_V"], np.float32)
    qk_read = np.asarray(inputs["qk_read"], np.float32)
    qk_write = np.asarray(inputs["qk_write"], np.float32)
    v_read = np.asarray(inputs["v_read"], np.float32)
    v_write = np.asarray(inputs["v_write"], np.float32)
    W_O = np.asarray(inputs["W_O"], np.float32)

    nc = _get_nc()
    qk_readT8 = np.ascontiguousarray(qk_read.T * QK_PRESCALE).astype(
        ml_dtypes.float8_e4m3)
    v_readT = _bf(v_read.T)
    # the in-kernel rowsum scale is bf16(0.81); fold the exact residual
    # (and the intended 1/0.81 dropout scaling) into W_O host-side
    wo_fix = W_O * (KEEP2_BF / 0.81)
    in_maps = []
    for c in range(8):
        b, hg = divmod(c, 4)
        cs = slice(hg * DHG, (hg + 1) * DHG)
        xbT = np.ascontiguousarray(x[b].T)
        in_maps.append({
            "xT": xbT.astype(ml_dtypes.bfloat16),
            "xT8": xbT.astype(ml_dtypes.float8_e4m3),
            "gqT": _bf(g_Q[b].T),
            "gkT": _bf(g_K[b].T),
            "gvT": _bf(g_V[b].T),
            "qk_readT8": qk_readT8,
            "v_readT": v_readT,
            "qk_write_hg": np.ascontiguousarray(
                qk_write[:, cs] * QK_PRESCALE).astype(ml_dtypes.float8_e4m3),
            "v_write_hg": _bf(v_write[:, cs]),
            "wo_cols": _bf(wo_fix[:, cs]),
        })
    res = run_bass_kernel_spmd(nc, in_maps, core_ids=list(range(8)))
    _CACHE['last_results'] = res
    out = np.empty((B, S, D), np.float32)
    for c in range(8):
        b, hg = divmod(c, 4)
        out[b, :, hg * DHG:(hg + 1) * DHG] = res.results[c]["out"]
    return out
